# revision 31
# baseline (speedup 1.0000x reference)
"""Trainium2 Bass kernel for nn_CMEncoder (cross-attention + LayerNorm2d + MLP block).

Strategy (8 NeuronCores, sequence-parallel over the HW=4096 query tokens):
  - Each core owns 512 query tokens; the 4096-token context is replicated.
  - Host-side algebraic folds remove the K and V projections entirely:
      scores^T = y^T (Wk^T Wq) x = y^T Qt        with Qt = G x,  G = Wk^T Wq
      z        = Wo Wv (y P) r + bo'             with B = Wo Wv
    so the score matmuls use y (channel-major) as the stationary operand and
    the P@V matmuls use y^T (token-major, uploaded separately) as stationary.
  - When bo' == 0 (true for zero attention biases), LayerNorm's per-column
    scale invariance absorbs the softmax denominator r = 1/colsum entirely:
    the colsum reduction matmuls and the reciprocal chain are dropped
    (zero_bias variant, selected at runtime from the actual inputs).
  - Everything stays channel-major on chip ([feature partition, token free]).
  - Input DMAs are spread over the two 16-engine DMA queues (sync/gpsimd;
    the scalar queue is single-engine and only carries tiny pieces) and
    ordered by first use so the attention loop is not DMA-paced.
  - A burst of junk matmuls during the initial DMA wait ramps the PE's HAM
    clock gate to full speed before the real work arrives.
  - Softmax exps run as paired [128,1024] ACT ops over two PSUM banks;
    the attention loop is software-pipelined by one pair so the exp latency
    stays off the PE's static instruction order.
  - The LayerNorm stat/apply chain is split into two column halves so its
    serial ACT/DVE latency pipelines across engines.
"""

import math
import numpy as np
import concourse.bacc as bacc
import concourse.mybir as mybir
import concourse.tile as tile
from concourse import bass_utils
from concourse.hw_specs import get_activation_tables

F32 = mybir.dt.float32
BF16 = mybir.dt.bfloat16
AF = mybir.ActivationFunctionType
ALU = mybir.AluOpType

MMDT = BF16      # matmul operand dtype

C = 256          # channels
HW = 4096        # query tokens (64x64)
NCTX = 4096      # context tokens
HID = 512        # mlp hidden
NCORES = 8
QS = HW // NCORES   # 512 queries per core
NBLK = NCTX // 128  # 32 context chunks
NPAIR = NBLK // 2   # 16 chunk pairs
EPS = 1e-6
NWARM = 6           # junk matmuls to ramp the PE clock during DMA wait

# context-chunk pieces for DMA (chunk ranges, sized so early data lands fast)
YPIECES = [(0, 4), (4, 8), (8, 16), (16, 24), (24, 32)]


def _build_nc(zero_bias):
    nc = bacc.Bacc("TRN2", target_bir_lowering=False)

    # --- DRAM I/O (packed on host; see prep_in_maps) ---
    d_x = nc.dram_tensor("x_mm", (128, 2 * QS), MMDT, kind="ExternalInput")
    d_g = nc.dram_tensor("g_mm", (128, 2 * C), MMDT, kind="ExternalInput")
    # y channel-major, piece p holds chunks [a,b): cols cc*(b-a)*128 + local
    d_y = [nc.dram_tensor(f"y{p}", (128, 2 * (b - a) * 128), MMDT,
                          kind="ExternalInput")
           for p, (a, b) in enumerate(YPIECES)]
    # y token-major, piece p: [128, (b-a)*C]
    d_yt = [nc.dram_tensor(f"yt{p}", (128, (b - a) * C), MMDT,
                           kind="ExternalInput")
            for p, (a, b) in enumerate(YPIECES)]
    # w: [b_t (2*C) | w1_t (2*HID) | w2_t (4*C)]
    d_w = nc.dram_tensor("w", (128, 2 * C + 2 * HID + 4 * C), MMDT,
                         kind="ExternalInput")
    # f32 smalls: [b1p (4) | bvec0 (3) | bvec1 (3)]
    d_f = nc.dram_tensor("fv", (128, 10), F32, kind="ExternalInput")
    d_out = nc.dram_tensor("out_sh", (C, QS), F32, kind="ExternalOutput")

    tabs = list(get_activation_tables(nc.m.arch).keys())
    LNEXP_SET = tabs.index("natural_log_exp_and_others")

    with tile.TileContext(nc) as tc:
        # Pre-load the exp+ln activation table once so the auto-inserted loads
        # don't ping-pong between exp-only and ln-only sets mid-kernel.
        nc.scalar.add_instruction(mybir.InstLoadActFuncSet(
            name=nc.get_next_instruction_name(), ins=[], outs=[],
            act_func_set_id=LNEXP_SET))

        with (
            tc.tile_pool(name="sb", bufs=1) as sb,
            tc.tile_pool(name="pt_pool", bufs=3) as ptp,
            tc.tile_pool(name="scp", bufs=2, space="PSUM") as scp,
            tc.tile_pool(name="ps", bufs=1, space="PSUM") as ps,
        ):
            # -------- input DMAs --------
            # sync and gpsimd are 16-engine striped queues (~150-300 GB/s);
            # the scalar queue is single-engine (~60 GB/s) -> tiny/late only.
            xm = [sb.tile([128, QS], MMDT, name=f"xm{i}") for i in range(2)]
            g_t = sb.tile([128, 2 * C], MMDT)
            yq = [sb.tile([128, 2 * (b - a) * 128], MMDT, name=f"yq{p}")
                  for p, (a, b) in enumerate(YPIECES)]
            ytt = [sb.tile([128, (b - a) * C], MMDT, name=f"ytt{p}")
                   for p, (a, b) in enumerate(YPIECES)]
            fv = sb.tile([128, 10], F32)
            wt = sb.tile([128, 2 * C + 2 * HID + 4 * C], MMDT)

            nc.sync.dma_start(xm[0], d_x[:, 0:QS])
            nc.sync.dma_start(yq[0], d_y[0][:, :])
            nc.sync.dma_start(ytt[0], d_yt[0][:, :])
            nc.sync.dma_start(yq[2], d_y[2][:, :])
            nc.sync.dma_start(ytt[2], d_yt[2][:, :])
            nc.scalar.dma_start(g_t, d_g[:, :])
            nc.scalar.dma_start(fv, d_f[:, :])
            nc.scalar.dma_start(ytt[4], d_yt[4][:, :])
            nc.gpsimd.dma_start(xm[1], d_x[:, QS:2 * QS])
            nc.gpsimd.dma_start(yq[1], d_y[1][:, :])
            nc.gpsimd.dma_start(ytt[1], d_yt[1][:, :])
            nc.gpsimd.dma_start(yq[3], d_y[3][:, :])
            nc.gpsimd.dma_start(yq[4], d_y[4][:, :])
            nc.gpsimd.dma_start(ytt[3], d_yt[3][:, :])
            nc.gpsimd.dma_start(wt, d_w[:, :])

            b_t = wt[:, 0:2 * C]
            w1_t = wt[:, 2 * C:2 * C + 2 * HID]
            w2_t = wt[:, 2 * C + 2 * HID:2 * C + 2 * HID + 4 * C]
            b1p = fv[:, 0:4]
            bvec = [fv[:, 4:7], fv[:, 7:10]]

            ones_c = sb.tile([128, 2], MMDT)
            nc.vector.memset(ones_c, 1.0)
            ones_r = sb.tile([1, 128], MMDT)
            nc.vector.memset(ones_r, 1.0)
            eps2v = sb.tile([1, 1], F32)
            nc.vector.memset(eps2v, float(C) * float(C) * EPS)
            lnCv = sb.tile([1, 1], F32)
            nc.vector.memset(lnCv, math.log(float(C)))
            junk = sb.tile([128, 512], MMDT)
            nc.vector.memset(junk, 0.0)

            # -------- PE clock-gate warmup on junk data during DMA wait ------
            for w in range(NWARM):
                wup = ps.tile([2, 512], F32, tag="st0", name=f"wup{w}")
                nc.tensor.matmul(wup, ones_c, junk, start=True, stop=True)

            def wsl(t, cc, cb, w=128):
                # packed weight tile slice: row-chunk cc, col-chunk cb
                return t[:, cc * (t.shape[1] // 2) + cb * w:
                         cc * (t.shape[1] // 2) + (cb + 1) * w]

            def piece(i):
                for p, (a, b) in enumerate(YPIECES):
                    if a <= i < b:
                        return p, a, b
                raise ValueError(i)

            def ysl(i, cc):
                # y channel-major stationary slice for context chunk i
                p, a, b = piece(i)
                col = cc * (b - a) * 128 + (i - a) * 128
                return yq[p][:, col:col + 128]

            def ytsl(i, cb):
                # y token-major stationary slice for context chunk i
                p, a, b = piece(i)
                col = (i - a) * C + cb * 128
                return ytt[p][:, col:col + 128]

            # ---------------- Qt = (G x + Wk^T bq)/16, channel-major ---------
            # (the 1/16 softmax scale is folded into G and qb on host)
            qp = [sb.tile([128, QS], MMDT, name=f"qp{i}") for i in range(2)]
            qps = scp.tile([128, 1024], F32, tag="spair", name="qps")
            for cb in range(2):
                nc.tensor.matmul(qps[:, cb * 512:(cb + 1) * 512],
                                 wsl(g_t, 0, cb), xm[0],
                                 start=True, stop=False)
                nc.tensor.matmul(qps[:, cb * 512:(cb + 1) * 512],
                                 wsl(g_t, 1, cb), xm[1],
                                 start=False, stop=True)
                if cb == 0:
                    nc.scalar.activation(qp[cb], qps[:, 0:512], AF.Identity,
                                         bias=bvec[cb][:, 0:1])
                else:
                    # DVE path so both qp copies overlap (junk is zeros)
                    nc.vector.scalar_tensor_tensor(qp[cb], qps[:, 512:1024],
                                                   bvec[cb][:, 0:1], junk,
                                                   op0=ALU.add, op1=ALU.add)

            # ---------------- attention state ----------------
            attps = [ps.tile([128, QS], F32, tag=f"attps{j}", bufs=1,
                             name=f"attps{j}") for j in range(2)]
            csum = None
            if not zero_bias:
                csum = ps.tile([2, QS], F32, tag="st1", bufs=1)

            att_s = [sb.tile([128, QS], MMDT, name=f"att_s{i}") for i in range(2)]
            zs = [sb.tile([128, QS], MMDT, name=f"zs{i}") for i in range(2)]
            zsq = [sb.tile([128, QS], MMDT, name=f"zsq{i}") for i in range(2)]
            zln = [sb.tile([128, QS], MMDT, name=f"zln{i}") for i in range(2)]
            hs = [sb.tile([128, QS], MMDT, name=f"hs{i}") for i in range(4)]
            ot = [sb.tile([128, QS], F32, name=f"ot{i}") for i in range(2)]
            rstd = sb.tile([1, QS], MMDT)
            nmrs = sb.tile([1, QS], MMDT)
            nmC = sb.tile([1, QS], F32)
            s2c = sb.tile([1, QS], F32)
            var = sb.tile([1, QS], F32)
            lnv = sb.tile([1, QS], F32)

            def attn_score(p):
                """S^T and exp for context chunk pair (2p, 2p+1)"""
                sps = scp.tile([128, 1024], F32, tag="spair", name=f"sps{p}")
                for k in range(2):
                    i = 2 * p + k
                    nc.tensor.matmul(sps[:, k * 512:(k + 1) * 512],
                                     ysl(i, 0), qp[0], start=True, stop=False)
                    nc.tensor.matmul(sps[:, k * 512:(k + 1) * 512],
                                     ysl(i, 1), qp[1], start=False, stop=True)
                pt = ptp.tile([128, 1024], MMDT, tag="pt", name=f"pt{p}")
                nc.scalar.activation(pt, sps, AF.Exp)
                return pt

            def attn_accum(p, pt):
                """(y P) [and colsum] accumulation for chunk pair (2p, 2p+1)"""
                for k in range(2):
                    i = 2 * p + k
                    first, last = (i == 0), (i == NBLK - 1)
                    pts = pt[:, k * 512:(k + 1) * 512]
                    for cb in range(2):
                        nc.tensor.matmul(attps[cb], ytsl(i, cb), pts,
                                         start=first, stop=last)
                    if not zero_bias:
                        nc.tensor.matmul(csum, ones_c, pts, start=first, stop=last)

            # ---- attention, software-pipelined by one chunk pair so the exp
            # ---- latency sits off the PE's static instruction order ----
            prev = attn_score(0)
            for p in range(1, NPAIR):
                cur = attn_score(p)
                attn_accum(p - 1, prev)
                prev = cur
            attn_accum(NPAIR - 1, prev)

            if zero_bias:
                # LN absorbs the softmax denominator: att_s = yP as-is.
                # Split the two copies across ACT and DVE so they overlap.
                nc.scalar.copy(att_s[0], attps[0])
                nc.vector.tensor_copy(att_s[1], attps[1])
            else:
                # softmax normalize: 1/colsum via exp(-ln(x)) on ACT
                lncs = sb.tile([1, QS], F32)
                rr = sb.tile([1, QS], MMDT)
                rb_s = sb.tile([128, QS], F32)
                nc.scalar.activation(lncs, csum[0:1, :], AF.Ln)
                nc.scalar.activation(rr, lncs, AF.Exp, scale=-1.0)
                rb = ps.tile([128, QS], F32, tag="st0", name="rb")
                nc.tensor.matmul(rb, ones_r, rr, start=True, stop=True)
                nc.vector.tensor_copy(rb_s, rb)
                for cb in range(2):
                    nc.vector.scalar_tensor_tensor(att_s[cb], attps[cb], 1.0,
                                                   rb_s, op0=ALU.mult,
                                                   op1=ALU.mult)

            # z = B @ att_s + bo'
            zps = scp.tile([128, 1024], F32, tag="spair", name="zps")
            for cb in range(2):
                nc.tensor.matmul(zps[:, cb * 512:(cb + 1) * 512],
                                 wsl(b_t, 0, cb), att_s[0], start=True, stop=False)
                nc.tensor.matmul(zps[:, cb * 512:(cb + 1) * 512],
                                 wsl(b_t, 1, cb), att_s[1], start=False, stop=True)

            HQ = QS // 2
            sls = [slice(0, HQ), slice(HQ, QS)]

            if zero_bias:
                # zs (DVE copy) and zsq (ACT square) per column half, so the
                # LN stat chain pipelines across engines
                for h in range(2):
                    for cb in range(2):
                        zph = zps[:, cb * 512 + h * HQ: cb * 512 + (h + 1) * HQ]
                        nc.vector.tensor_copy(zs[cb][:, sls[h]], zph)
                        nc.vector.tensor_mul(zsq[cb][:, sls[h]],
                                             zs[cb][:, sls[h]],
                                             zs[cb][:, sls[h]])
            else:
                for cb in range(2):
                    zph = zps[:, cb * 512:(cb + 1) * 512]
                    nc.scalar.activation(zs[cb], zph, AF.Identity,
                                         bias=bvec[cb][:, 1:2])
                    nc.vector.tensor_mul(zsq[cb], zs[cb], zs[cb])

            # LN stats per column half in the freed attps banks
            szp, sqp = {}, {}
            for h in range(2):
                szp[h] = ps.tile([2, HQ], F32, tag="attps0", name=f"szp{h}")
                nc.tensor.matmul(szp[h], ones_c, zs[0][:, sls[h]],
                                 start=True, stop=False)
                nc.tensor.matmul(szp[h], ones_c, zs[1][:, sls[h]],
                                 start=False, stop=True)
                sqp[h] = ps.tile([2, HQ], F32, tag="attps1", name=f"sqp{h}")
                nc.tensor.matmul(sqp[h], ones_c, zsq[0][:, sls[h]],
                                 start=True, stop=False)
                nc.tensor.matmul(sqp[h], ones_c, zsq[1][:, sls[h]],
                                 start=False, stop=True)

            # var*C^2 = C*sum(z^2) - (sum z)^2; rstd = C/sqrt(var*C^2 + C^2 eps)
            for h in range(2):
                sl = sls[h]
                nc.vector.tensor_scalar_mul(nmC[:, sl], szp[h][0:1, :], -1.0)
                nc.vector.tensor_mul(s2c[:, sl], nmC[:, sl], nmC[:, sl])
                nc.vector.scalar_tensor_tensor(var[:, sl], sqp[h][0:1, :],
                                               float(C), s2c[:, sl],
                                               op0=ALU.mult, op1=ALU.subtract)
                nc.scalar.activation(lnv[:, sl], var[:, sl], AF.Ln, bias=eps2v)
                nc.scalar.activation(rstd[:, sl], lnv[:, sl], AF.Exp,
                                     scale=-0.5, bias=lnCv)
                nc.vector.scalar_tensor_tensor(nmrs[:, sl], nmC[:, sl], 1.0 / C,
                                               rstd[:, sl], op0=ALU.mult,
                                               op1=ALU.mult)

            zt = [sb.tile([128, QS], MMDT, name=f"zt{cb}") for cb in range(2)]
            for h in range(2):
                sl = sls[h]
                rstd_b = ps.tile([128, HQ], F32, tag="attps0", name=f"rstdb{h}")
                nc.tensor.matmul(rstd_b, ones_r, rstd[:, sl], start=True, stop=True)
                nmrs_b = ps.tile([128, HQ], F32, tag="attps1", name=f"nmrsb{h}")
                nc.tensor.matmul(nmrs_b, ones_r, nmrs[:, sl], start=True, stop=True)
                for cb in range(2):
                    nc.vector.tensor_mul(zt[cb][:, sl], zs[cb][:, sl], rstd_b)
                    nc.vector.tensor_add(zln[cb][:, sl], zt[cb][:, sl], nmrs_b)

            # MLP + residual
            for hp in range(2):
                hps = scp.tile([128, 1024], F32, tag="spair", name=f"hps{hp}")
                for k in range(2):
                    hb = 2 * hp + k
                    nc.tensor.matmul(hps[:, k * 512:(k + 1) * 512],
                                     wsl(w1_t, 0, hb), zln[0], start=True, stop=False)
                    nc.tensor.matmul(hps[:, k * 512:(k + 1) * 512],
                                     wsl(w1_t, 1, hb), zln[1], start=False, stop=True)
                    nc.scalar.activation(hs[hb], hps[:, k * 512:(k + 1) * 512],
                                         AF.Gelu, bias=b1p[:, hb:hb + 1])

            tps = scp.tile([128, 1024], F32, tag="spair", name="tps")
            for cb in range(2):
                for hb in range(4):
                    nc.tensor.matmul(
                        tps[:, cb * 512:(cb + 1) * 512],
                        w2_t[:, hb * 256 + cb * 128:hb * 256 + (cb + 1) * 128],
                        hs[hb], start=(hb == 0), stop=(hb == 3))
            # out = mlp + b2 + residual, quartered so DMA-out starts early
            for h in range(2):
                for cb in range(2):
                    nc.vector.scalar_tensor_tensor(
                        ot[cb][:, sls[h]],
                        tps[:, cb * 512 + h * HQ: cb * 512 + (h + 1) * HQ],
                        bvec[cb][:, 2:3], xm[cb][:, h * HQ:
                                              (h + 1) * HQ],
                        op0=ALU.add, op1=ALU.add)
                    q = nc.sync if cb == 0 else nc.gpsimd
                    q.dma_start(d_out[cb * 128:(cb + 1) * 128, h * HQ:(h + 1) * HQ],
                                ot[cb][:, sls[h]])

    nc.compile()
    return nc


_NCS = {}


def _get_nc(zero_bias=True):
    if zero_bias not in _NCS:
        _NCS[zero_bias] = _build_nc(zero_bias)
    return _NCS[zero_bias]


def _pack_rows(a, nchunk):
    """(nchunk*128, W) -> (128, nchunk*W) with row-chunks side by side."""
    w = a.shape[1]
    out = np.empty((128, nchunk * w), a.dtype)
    for i in range(nchunk):
        out[:, i * w:(i + 1) * w] = a[i * 128:(i + 1) * 128, :]
    return out


def prep_in_maps(x, y, Wq, bq, Wk, bk, Wv, bv, Wo, bo, ln_w, ln_b, W1, b1, W2, b2):
    f = lambda a: np.asarray(a, dtype=np.float32)
    x, y = f(x), f(y)
    Wq, bq, Wk, Wv, bv, Wo, bo = f(Wq), f(bq), f(Wk), f(Wv), f(bv), f(Wo), f(bo)
    ln_w, ln_b, W1, b1, W2, b2 = f(ln_w), f(ln_b), f(W1), f(b1), f(W2), f(b2)

    mmnp = mybir.dt.np(MMDT)
    g = lambda a: np.ascontiguousarray(a).astype(mmnp)

    x_cm = np.ascontiguousarray(x.reshape(C, HW))
    y_cm = np.ascontiguousarray(y.reshape(C, NCTX))

    # host-side algebraic folds
    G = (Wk.astype(np.float64).T @ Wq.astype(np.float64) / 16.0).astype(np.float32)
    B = (Wo.astype(np.float64) @ Wv.astype(np.float64)).astype(np.float32)
    qb = (Wk.astype(np.float64).T @ bq.astype(np.float64)).astype(np.float32)
    bo_p = (Wo.astype(np.float64) @ bv.astype(np.float64) + bo).astype(np.float32)
    b1_p = (W1.astype(np.float64) @ ln_b.astype(np.float64) + b1).astype(np.float32)
    W1p = (W1 * ln_w[None, :]).astype(np.float32)

    # f32 smalls: [b1p (4) | bvec0 (3) | bvec1 (3)]
    bvec = np.stack([qb / 16.0, bo_p, b2], axis=1).astype(np.float32)  # (256,3)
    fv = np.concatenate([np.ascontiguousarray(b1_p.reshape(4, 128).T),
                         bvec[0:128, :], bvec[128:256, :]], axis=1)

    # y pieces, channel-major: piece p = chunks [a,b): [cc | chunk-local]
    y2 = _pack_rows(y_cm, 2)  # (128, 2*NCTX), cc side by side
    y_pieces = []
    for a, b in YPIECES:
        y_pieces.append(np.concatenate(
            [y2[:, cc * NCTX + a * 128: cc * NCTX + b * 128] for cc in range(2)],
            axis=1))
    # y pieces, token-major: chunk ci = y^T rows [ci*128,(ci+1)*128) = [128, C]
    y_tm = _pack_rows(np.ascontiguousarray(y_cm.T), NBLK)  # (128, NBLK*C)
    yt_pieces = [y_tm[:, a * C: b * C] for a, b in YPIECES]

    wpk = np.concatenate([_pack_rows(B.T, 2), _pack_rows(W1p.T, 2),
                          _pack_rows(W2.T, 4)], axis=1)

    common = {"w": g(wpk), "fv": fv.astype(np.float32),
              "g_mm": g(_pack_rows(G.T, 2))}
    for p in range(len(YPIECES)):
        common[f"y{p}"] = g(y_pieces[p])
        common[f"yt{p}"] = g(yt_pieces[p])

    in_maps = []
    for i in range(NCORES):
        m = dict(common)
        xs = np.ascontiguousarray(x_cm[:, i * QS:(i + 1) * QS])
        m["x_mm"] = g(_pack_rows(xs, 2))
        in_maps.append(m)
    return in_maps


def kernel(**inputs):
    in_maps = prep_in_maps(**inputs)
    f64 = lambda a: np.asarray(a, dtype=np.float64)
    bo_p = f64(inputs["Wo"]) @ f64(inputs["bv"]) + f64(inputs["bo"])
    nc = _get_nc(zero_bias=bool(np.abs(bo_p).max() == 0.0))
    res = bass_utils.run_bass_kernel_spmd(nc, in_maps, core_ids=list(range(NCORES)))
    t = np.concatenate([res.results[i]["out_sh"] for i in range(NCORES)], axis=1)
    return t.reshape(1, C, 64, 64)


# revision 33
# speedup vs baseline: 1.0132x; 1.0132x over previous
"""Trainium2 Bass kernel for nn_CMEncoder (cross-attention + LayerNorm2d + MLP block).

Strategy (8 NeuronCores, sequence-parallel over the HW=4096 query tokens):
  - Each core owns 512 query tokens; the 4096-token context is replicated.
  - Host-side algebraic folds remove the K and V projections entirely:
      scores^T = y^T (Wk^T Wq) x = y^T Qt        with Qt = G x,  G = Wk^T Wq
      z        = Wo Wv (y P) r + bo'             with B = Wo Wv
    so the score matmuls use y (channel-major) as the stationary operand and
    the P@V matmuls use y^T (token-major, uploaded separately) as stationary.
  - When bo' == 0 (true for zero attention biases), LayerNorm's per-column
    scale invariance absorbs the softmax denominator r = 1/colsum entirely:
    the colsum reduction matmuls and the reciprocal chain are dropped
    (zero_bias variant, selected at runtime from the actual inputs).
  - Everything stays channel-major on chip ([feature partition, token free]).
  - Input DMAs are spread over the two 16-engine DMA queues (sync/gpsimd;
    the scalar queue is single-engine and only carries tiny pieces) and
    ordered by first use so the attention loop is not DMA-paced.
  - A burst of junk matmuls during the initial DMA wait ramps the PE's HAM
    clock gate to full speed before the real work arrives.
  - Softmax exps run as paired [128,1024] ACT ops over two PSUM banks;
    the attention loop is software-pipelined by one pair so the exp latency
    stays off the PE's static instruction order.
  - The LayerNorm stat/apply chain is split into two column halves so its
    serial ACT/DVE latency pipelines across engines.
"""

import math
import numpy as np
import concourse.bacc as bacc
import concourse.mybir as mybir
import concourse.tile as tile
from concourse import bass_utils
from concourse.hw_specs import get_activation_tables

F32 = mybir.dt.float32
BF16 = mybir.dt.bfloat16
AF = mybir.ActivationFunctionType
ALU = mybir.AluOpType

MMDT = BF16      # matmul operand dtype

C = 256          # channels
HW = 4096        # query tokens (64x64)
NCTX = 4096      # context tokens
HID = 512        # mlp hidden
NCORES = 8
QS = HW // NCORES   # 512 queries per core
NBLK = NCTX // 128  # 32 context chunks
NPAIR = NBLK // 2   # 16 chunk pairs
EPS = 1e-6
NWARM = 8           # junk matmuls to ramp the PE clock during DMA wait

# context-chunk pieces for DMA (chunk ranges, sized so early data lands fast)
YPIECES = [(0, 4), (4, 8), (8, 16), (16, 24), (24, 32)]


def _build_nc(zero_bias):
    nc = bacc.Bacc("TRN2", target_bir_lowering=False)

    # --- DRAM I/O (packed on host; see prep_in_maps) ---
    d_x = nc.dram_tensor("x_mm", (128, 2 * QS), MMDT, kind="ExternalInput")
    d_g = nc.dram_tensor("g_mm", (128, 2 * C), MMDT, kind="ExternalInput")
    # y channel-major, piece p holds chunks [a,b): cols cc*(b-a)*128 + local
    d_y = [nc.dram_tensor(f"y{p}", (128, 2 * (b - a) * 128), MMDT,
                          kind="ExternalInput")
           for p, (a, b) in enumerate(YPIECES)]
    # y token-major, piece p: [128, (b-a)*C]
    d_yt = [nc.dram_tensor(f"yt{p}", (128, (b - a) * C), MMDT,
                           kind="ExternalInput")
            for p, (a, b) in enumerate(YPIECES)]
    # w: [b_t (2*C) | w1_t (2*HID) | w2_t (4*C)]
    d_w = nc.dram_tensor("w", (128, 2 * C + 2 * HID + 4 * C), MMDT,
                         kind="ExternalInput")
    # f32 smalls: [b1p (4) | bvec0 (3) | bvec1 (3)]
    d_f = nc.dram_tensor("fv", (128, 10), F32, kind="ExternalInput")
    d_out = nc.dram_tensor("out_sh", (C, QS), F32, kind="ExternalOutput")

    tabs = list(get_activation_tables(nc.m.arch).keys())
    LNEXP_SET = tabs.index("natural_log_exp_and_others")

    with tile.TileContext(nc) as tc:
        with (
            tc.tile_pool(name="sb", bufs=1) as sb,
            tc.tile_pool(name="pt_pool", bufs=3) as ptp,
            tc.tile_pool(name="scp", bufs=2, space="PSUM") as scp,
            tc.tile_pool(name="ps", bufs=1, space="PSUM") as ps,
        ):
            # -------- input DMAs --------
            # sync and gpsimd are 16-engine striped queues (~150-300 GB/s);
            # the scalar queue is single-engine (~60 GB/s) -> tiny/late only.
            xm = [sb.tile([128, QS], MMDT, name=f"xm{i}") for i in range(2)]
            g_t = sb.tile([128, 2 * C], MMDT)
            yq = [sb.tile([128, 2 * (b - a) * 128], MMDT, name=f"yq{p}")
                  for p, (a, b) in enumerate(YPIECES)]
            ytt = [sb.tile([128, (b - a) * C], MMDT, name=f"ytt{p}")
                   for p, (a, b) in enumerate(YPIECES)]
            fv = sb.tile([128, 10], F32)
            wt = sb.tile([128, 2 * C + 2 * HID + 4 * C], MMDT)

            nc.sync.dma_start(xm[0], d_x[:, 0:QS])
            nc.sync.dma_start(yq[0], d_y[0][:, :])
            nc.sync.dma_start(ytt[0], d_yt[0][:, :])
            nc.sync.dma_start(yq[2], d_y[2][:, :])
            nc.sync.dma_start(ytt[2], d_yt[2][:, :])
            nc.scalar.dma_start(g_t, d_g[:, :])
            nc.scalar.dma_start(fv, d_f[:, :])
            nc.scalar.dma_start(ytt[4], d_yt[4][:, :])
            nc.gpsimd.dma_start(xm[1], d_x[:, QS:2 * QS])
            nc.gpsimd.dma_start(yq[1], d_y[1][:, :])
            nc.gpsimd.dma_start(ytt[1], d_yt[1][:, :])
            nc.gpsimd.dma_start(yq[3], d_y[3][:, :])
            nc.gpsimd.dma_start(yq[4], d_y[4][:, :])
            nc.gpsimd.dma_start(ytt[3], d_yt[3][:, :])
            nc.gpsimd.dma_start(wt, d_w[:, :])

            # Pre-load the exp+ln activation table once so the auto-inserted
            # loads don't ping-pong between sets mid-kernel. Injected after
            # the DMA triggers so it doesn't delay them on the scalar queue
            # (the table is only needed by the first exp, ~16us in).
            nc.scalar.add_instruction(mybir.InstLoadActFuncSet(
                name=nc.get_next_instruction_name(), ins=[], outs=[],
                act_func_set_id=LNEXP_SET))

            b_t = wt[:, 0:2 * C]
            w1_t = wt[:, 2 * C:2 * C + 2 * HID]
            w2_t = wt[:, 2 * C + 2 * HID:2 * C + 2 * HID + 4 * C]
            b1p = fv[:, 0:4]
            bvec = [fv[:, 4:7], fv[:, 7:10]]

            ones_c = sb.tile([128, 2], MMDT)
            nc.vector.memset(ones_c, 1.0)
            ones_r = sb.tile([1, 128], MMDT)
            nc.vector.memset(ones_r, 1.0)
            eps2v = sb.tile([1, 1], F32)
            nc.vector.memset(eps2v, float(C) * float(C) * EPS)
            lnCv = sb.tile([1, 1], F32)
            nc.vector.memset(lnCv, math.log(float(C)))
            junk = sb.tile([128, 512], MMDT)
            nc.vector.memset(junk, 0.0)

            # -------- PE clock-gate warmup on junk data during DMA wait ------
            for w in range(NWARM):
                wup = ps.tile([2, 512], F32, tag="st0", name=f"wup{w}")
                nc.tensor.matmul(wup, ones_c, junk, start=True, stop=True)

            def wsl(t, cc, cb, w=128):
                # packed weight tile slice: row-chunk cc, col-chunk cb
                return t[:, cc * (t.shape[1] // 2) + cb * w:
                         cc * (t.shape[1] // 2) + (cb + 1) * w]

            def piece(i):
                for p, (a, b) in enumerate(YPIECES):
                    if a <= i < b:
                        return p, a, b
                raise ValueError(i)

            def ysl(i, cc):
                # y channel-major stationary slice for context chunk i
                p, a, b = piece(i)
                col = cc * (b - a) * 128 + (i - a) * 128
                return yq[p][:, col:col + 128]

            def ytsl(i, cb):
                # y token-major stationary slice for context chunk i
                p, a, b = piece(i)
                col = (i - a) * C + cb * 128
                return ytt[p][:, col:col + 128]

            # ---------------- Qt = (G x + Wk^T bq)/16, channel-major ---------
            # (the 1/16 softmax scale is folded into G and qb on host)
            qp = [sb.tile([128, QS], MMDT, name=f"qp{i}") for i in range(2)]
            qps = scp.tile([128, 1024], F32, tag="spair", name="qps")
            for cb in range(2):
                nc.tensor.matmul(qps[:, cb * 512:(cb + 1) * 512],
                                 wsl(g_t, 0, cb), xm[0],
                                 start=True, stop=False)
                nc.tensor.matmul(qps[:, cb * 512:(cb + 1) * 512],
                                 wsl(g_t, 1, cb), xm[1],
                                 start=False, stop=True)
                if cb == 0:
                    nc.scalar.activation(qp[cb], qps[:, 0:512], AF.Identity,
                                         bias=bvec[cb][:, 0:1])
                else:
                    # DVE path so both qp copies overlap (junk is zeros)
                    nc.vector.scalar_tensor_tensor(qp[cb], qps[:, 512:1024],
                                                   bvec[cb][:, 0:1], junk,
                                                   op0=ALU.add, op1=ALU.add)

            # ---------------- attention state ----------------
            attps = [ps.tile([128, QS], F32, tag=f"attps{j}", bufs=1,
                             name=f"attps{j}") for j in range(2)]
            csum = None
            if not zero_bias:
                csum = ps.tile([2, QS], F32, tag="st1", bufs=1)

            att_s = [sb.tile([128, QS], MMDT, name=f"att_s{i}") for i in range(2)]
            zs = [sb.tile([128, QS], MMDT, name=f"zs{i}") for i in range(2)]
            zsq = [sb.tile([128, QS], MMDT, name=f"zsq{i}") for i in range(2)]
            zln = [sb.tile([128, QS], MMDT, name=f"zln{i}") for i in range(2)]
            hs = [sb.tile([128, QS], MMDT, name=f"hs{i}") for i in range(4)]
            ot = [sb.tile([128, QS], F32, name=f"ot{i}") for i in range(2)]
            rstd = sb.tile([1, QS], MMDT)
            nmrs = sb.tile([1, QS], MMDT)
            nmC = sb.tile([1, QS], F32)
            s2c = sb.tile([1, QS], F32)
            var = sb.tile([1, QS], F32)
            lnv = sb.tile([1, QS], F32)

            def attn_score(p):
                """S^T and exp for context chunk pair (2p, 2p+1)"""
                sps = scp.tile([128, 1024], F32, tag="spair", name=f"sps{p}")
                for k in range(2):
                    i = 2 * p + k
                    nc.tensor.matmul(sps[:, k * 512:(k + 1) * 512],
                                     ysl(i, 0), qp[0], start=True, stop=False)
                    nc.tensor.matmul(sps[:, k * 512:(k + 1) * 512],
                                     ysl(i, 1), qp[1], start=False, stop=True)
                pt = ptp.tile([128, 1024], MMDT, tag="pt", name=f"pt{p}")
                nc.scalar.activation(pt, sps, AF.Exp)
                return pt

            def attn_accum(p, pt):
                """(y P) [and colsum] accumulation for chunk pair (2p, 2p+1)"""
                for k in range(2):
                    i = 2 * p + k
                    first, last = (i == 0), (i == NBLK - 1)
                    pts = pt[:, k * 512:(k + 1) * 512]
                    for cb in range(2):
                        nc.tensor.matmul(attps[cb], ytsl(i, cb), pts,
                                         start=first, stop=last)
                    if not zero_bias:
                        nc.tensor.matmul(csum, ones_c, pts, start=first, stop=last)

            # ---- attention, software-pipelined by one chunk pair so the exp
            # ---- latency sits off the PE's static instruction order ----
            prev = attn_score(0)
            for p in range(1, NPAIR):
                cur = attn_score(p)
                attn_accum(p - 1, prev)
                prev = cur
            attn_accum(NPAIR - 1, prev)

            if zero_bias:
                # LN absorbs the softmax denominator: att_s = yP as-is.
                # Split the two copies across ACT and DVE so they overlap.
                nc.scalar.copy(att_s[0], attps[0])
                nc.vector.tensor_copy(att_s[1], attps[1])
            else:
                # softmax normalize: 1/colsum via exp(-ln(x)) on ACT
                lncs = sb.tile([1, QS], F32)
                rr = sb.tile([1, QS], MMDT)
                rb_s = sb.tile([128, QS], F32)
                nc.scalar.activation(lncs, csum[0:1, :], AF.Ln)
                nc.scalar.activation(rr, lncs, AF.Exp, scale=-1.0)
                rb = ps.tile([128, QS], F32, tag="st0", name="rb")
                nc.tensor.matmul(rb, ones_r, rr, start=True, stop=True)
                nc.vector.tensor_copy(rb_s, rb)
                for cb in range(2):
                    nc.vector.scalar_tensor_tensor(att_s[cb], attps[cb], 1.0,
                                                   rb_s, op0=ALU.mult,
                                                   op1=ALU.mult)

            # z = B @ att_s + bo'
            zps = scp.tile([128, 1024], F32, tag="spair", name="zps")
            for cb in range(2):
                nc.tensor.matmul(zps[:, cb * 512:(cb + 1) * 512],
                                 wsl(b_t, 0, cb), att_s[0], start=True, stop=False)
                nc.tensor.matmul(zps[:, cb * 512:(cb + 1) * 512],
                                 wsl(b_t, 1, cb), att_s[1], start=False, stop=True)

            HQ = QS // 2
            sls = [slice(0, HQ), slice(HQ, QS)]

            if zero_bias:
                # zs (DVE copy) and zsq (ACT square) per column half, so the
                # LN stat chain pipelines across engines
                for h in range(2):
                    for cb in range(2):
                        zph = zps[:, cb * 512 + h * HQ: cb * 512 + (h + 1) * HQ]
                        nc.vector.tensor_copy(zs[cb][:, sls[h]], zph)
                        nc.vector.tensor_mul(zsq[cb][:, sls[h]],
                                             zs[cb][:, sls[h]],
                                             zs[cb][:, sls[h]])
            else:
                for cb in range(2):
                    zph = zps[:, cb * 512:(cb + 1) * 512]
                    nc.scalar.activation(zs[cb], zph, AF.Identity,
                                         bias=bvec[cb][:, 1:2])
                    nc.vector.tensor_mul(zsq[cb], zs[cb], zs[cb])

            # LN stats per column half in the freed attps banks
            szp, sqp = {}, {}
            for h in range(2):
                szp[h] = ps.tile([2, HQ], F32, tag="attps0", name=f"szp{h}")
                nc.tensor.matmul(szp[h], ones_c, zs[0][:, sls[h]],
                                 start=True, stop=False)
                nc.tensor.matmul(szp[h], ones_c, zs[1][:, sls[h]],
                                 start=False, stop=True)
                sqp[h] = ps.tile([2, HQ], F32, tag="attps1", name=f"sqp{h}")
                nc.tensor.matmul(sqp[h], ones_c, zsq[0][:, sls[h]],
                                 start=True, stop=False)
                nc.tensor.matmul(sqp[h], ones_c, zsq[1][:, sls[h]],
                                 start=False, stop=True)

            # var*C^2 = C*sum(z^2) - (sum z)^2; rstd = C/sqrt(var*C^2 + C^2 eps)
            for h in range(2):
                sl = sls[h]
                nc.vector.tensor_scalar_mul(nmC[:, sl], szp[h][0:1, :], -1.0)
                nc.vector.tensor_mul(s2c[:, sl], nmC[:, sl], nmC[:, sl])
                nc.vector.scalar_tensor_tensor(var[:, sl], sqp[h][0:1, :],
                                               float(C), s2c[:, sl],
                                               op0=ALU.mult, op1=ALU.subtract)
                nc.scalar.activation(lnv[:, sl], var[:, sl], AF.Ln, bias=eps2v)
                nc.scalar.activation(rstd[:, sl], lnv[:, sl], AF.Exp,
                                     scale=-0.5, bias=lnCv)
                nc.vector.scalar_tensor_tensor(nmrs[:, sl], nmC[:, sl], 1.0 / C,
                                               rstd[:, sl], op0=ALU.mult,
                                               op1=ALU.mult)

            zt = [sb.tile([128, QS], MMDT, name=f"zt{cb}") for cb in range(2)]
            for h in range(2):
                sl = sls[h]
                rstd_b = ps.tile([128, HQ], F32, tag="attps0", name=f"rstdb{h}")
                nc.tensor.matmul(rstd_b, ones_r, rstd[:, sl], start=True, stop=True)
                nmrs_b = ps.tile([128, HQ], F32, tag="attps1", name=f"nmrsb{h}")
                nc.tensor.matmul(nmrs_b, ones_r, nmrs[:, sl], start=True, stop=True)
                for cb in range(2):
                    nc.vector.tensor_mul(zt[cb][:, sl], zs[cb][:, sl], rstd_b)
                    nc.vector.tensor_add(zln[cb][:, sl], zt[cb][:, sl], nmrs_b)

            # MLP + residual
            for hp in range(2):
                hps = scp.tile([128, 1024], F32, tag="spair", name=f"hps{hp}")
                for k in range(2):
                    hb = 2 * hp + k
                    nc.tensor.matmul(hps[:, k * 512:(k + 1) * 512],
                                     wsl(w1_t, 0, hb), zln[0], start=True, stop=False)
                    nc.tensor.matmul(hps[:, k * 512:(k + 1) * 512],
                                     wsl(w1_t, 1, hb), zln[1], start=False, stop=True)
                    nc.scalar.activation(hs[hb], hps[:, k * 512:(k + 1) * 512],
                                         AF.Gelu, bias=b1p[:, hb:hb + 1])

            tps = scp.tile([128, 1024], F32, tag="spair", name="tps")
            for cb in range(2):
                for hb in range(4):
                    nc.tensor.matmul(
                        tps[:, cb * 512:(cb + 1) * 512],
                        w2_t[:, hb * 256 + cb * 128:hb * 256 + (cb + 1) * 128],
                        hs[hb], start=(hb == 0), stop=(hb == 3))
            # out = mlp + b2 + residual, quartered so DMA-out starts early
            for h in range(2):
                for cb in range(2):
                    nc.vector.scalar_tensor_tensor(
                        ot[cb][:, sls[h]],
                        tps[:, cb * 512 + h * HQ: cb * 512 + (h + 1) * HQ],
                        bvec[cb][:, 2:3], xm[cb][:, h * HQ:
                                              (h + 1) * HQ],
                        op0=ALU.add, op1=ALU.add)
                    q = nc.sync if cb == 0 else nc.gpsimd
                    q.dma_start(d_out[cb * 128:(cb + 1) * 128, h * HQ:(h + 1) * HQ],
                                ot[cb][:, sls[h]])

    nc.compile()
    return nc


_NCS = {}


def _get_nc(zero_bias=True):
    if zero_bias not in _NCS:
        _NCS[zero_bias] = _build_nc(zero_bias)
    return _NCS[zero_bias]


def _pack_rows(a, nchunk):
    """(nchunk*128, W) -> (128, nchunk*W) with row-chunks side by side."""
    w = a.shape[1]
    out = np.empty((128, nchunk * w), a.dtype)
    for i in range(nchunk):
        out[:, i * w:(i + 1) * w] = a[i * 128:(i + 1) * 128, :]
    return out


def prep_in_maps(x, y, Wq, bq, Wk, bk, Wv, bv, Wo, bo, ln_w, ln_b, W1, b1, W2, b2):
    f = lambda a: np.asarray(a, dtype=np.float32)
    x, y = f(x), f(y)
    Wq, bq, Wk, Wv, bv, Wo, bo = f(Wq), f(bq), f(Wk), f(Wv), f(bv), f(Wo), f(bo)
    ln_w, ln_b, W1, b1, W2, b2 = f(ln_w), f(ln_b), f(W1), f(b1), f(W2), f(b2)

    mmnp = mybir.dt.np(MMDT)
    g = lambda a: np.ascontiguousarray(a).astype(mmnp)

    x_cm = np.ascontiguousarray(x.reshape(C, HW))
    y_cm = np.ascontiguousarray(y.reshape(C, NCTX))

    # host-side algebraic folds
    G = (Wk.astype(np.float64).T @ Wq.astype(np.float64) / 16.0).astype(np.float32)
    B = (Wo.astype(np.float64) @ Wv.astype(np.float64)).astype(np.float32)
    qb = (Wk.astype(np.float64).T @ bq.astype(np.float64)).astype(np.float32)
    bo_p = (Wo.astype(np.float64) @ bv.astype(np.float64) + bo).astype(np.float32)
    b1_p = (W1.astype(np.float64) @ ln_b.astype(np.float64) + b1).astype(np.float32)
    W1p = (W1 * ln_w[None, :]).astype(np.float32)

    # f32 smalls: [b1p (4) | bvec0 (3) | bvec1 (3)]
    bvec = np.stack([qb / 16.0, bo_p, b2], axis=1).astype(np.float32)  # (256,3)
    fv = np.concatenate([np.ascontiguousarray(b1_p.reshape(4, 128).T),
                         bvec[0:128, :], bvec[128:256, :]], axis=1)

    # y pieces, channel-major: piece p = chunks [a,b): [cc | chunk-local]
    y2 = _pack_rows(y_cm, 2)  # (128, 2*NCTX), cc side by side
    y_pieces = []
    for a, b in YPIECES:
        y_pieces.append(np.concatenate(
            [y2[:, cc * NCTX + a * 128: cc * NCTX + b * 128] for cc in range(2)],
            axis=1))
    # y pieces, token-major: chunk ci = y^T rows [ci*128,(ci+1)*128) = [128, C]
    y_tm = _pack_rows(np.ascontiguousarray(y_cm.T), NBLK)  # (128, NBLK*C)
    yt_pieces = [y_tm[:, a * C: b * C] for a, b in YPIECES]

    wpk = np.concatenate([_pack_rows(B.T, 2), _pack_rows(W1p.T, 2),
                          _pack_rows(W2.T, 4)], axis=1)

    common = {"w": g(wpk), "fv": fv.astype(np.float32),
              "g_mm": g(_pack_rows(G.T, 2))}
    for p in range(len(YPIECES)):
        common[f"y{p}"] = g(y_pieces[p])
        common[f"yt{p}"] = g(yt_pieces[p])

    in_maps = []
    for i in range(NCORES):
        m = dict(common)
        xs = np.ascontiguousarray(x_cm[:, i * QS:(i + 1) * QS])
        m["x_mm"] = g(_pack_rows(xs, 2))
        in_maps.append(m)
    return in_maps


def kernel(**inputs):
    in_maps = prep_in_maps(**inputs)
    f64 = lambda a: np.asarray(a, dtype=np.float64)
    bo_p = f64(inputs["Wo"]) @ f64(inputs["bv"]) + f64(inputs["bo"])
    nc = _get_nc(zero_bias=bool(np.abs(bo_p).max() == 0.0))
    res = bass_utils.run_bass_kernel_spmd(nc, in_maps, core_ids=list(range(NCORES)))
    t = np.concatenate([res.results[i]["out_sh"] for i in range(NCORES)], axis=1)
    return t.reshape(1, C, 64, 64)


# revision 34
# speedup vs baseline: 1.0176x; 1.0043x over previous
"""Trainium2 Bass kernel for nn_CMEncoder (cross-attention + LayerNorm2d + MLP block).

Strategy (8 NeuronCores, sequence-parallel over the HW=4096 query tokens):
  - Each core owns 512 query tokens; the 4096-token context is replicated.
  - Host-side algebraic folds remove the K and V projections entirely:
      scores^T = y^T (Wk^T Wq) x = y^T Qt        with Qt = G x,  G = Wk^T Wq
      z        = Wo Wv (y P) r + bo'             with B = Wo Wv
    so the score matmuls use y (channel-major) as the stationary operand and
    the P@V matmuls use y^T (token-major, uploaded separately) as stationary.
  - When bo' == 0 (true for zero attention biases), LayerNorm's per-column
    scale invariance absorbs the softmax denominator r = 1/colsum entirely:
    the colsum reduction matmuls and the reciprocal chain are dropped
    (zero_bias variant, selected at runtime from the actual inputs).
  - Everything stays channel-major on chip ([feature partition, token free]).
  - Input DMAs are spread over the two 16-engine DMA queues (sync/gpsimd;
    the scalar queue is single-engine and only carries tiny pieces) and
    ordered by first use so the attention loop is not DMA-paced.
  - A burst of junk matmuls during the initial DMA wait ramps the PE's HAM
    clock gate to full speed before the real work arrives.
  - Softmax exps run as paired [128,1024] ACT ops over two PSUM banks;
    the attention loop is software-pipelined by one pair so the exp latency
    stays off the PE's static instruction order.
  - The LayerNorm stat/apply chain is split into two column halves so its
    serial ACT/DVE latency pipelines across engines.
"""

import math
import numpy as np
import concourse.bacc as bacc
import concourse.mybir as mybir
import concourse.tile as tile
from concourse import bass_utils
from concourse.hw_specs import get_activation_tables

F32 = mybir.dt.float32
BF16 = mybir.dt.bfloat16
AF = mybir.ActivationFunctionType
ALU = mybir.AluOpType

MMDT = BF16      # matmul operand dtype

C = 256          # channels
HW = 4096        # query tokens (64x64)
NCTX = 4096      # context tokens
HID = 512        # mlp hidden
NCORES = 8
QS = HW // NCORES   # 512 queries per core
NBLK = NCTX // 128  # 32 context chunks
NPAIR = NBLK // 2   # 16 chunk pairs
EPS = 1e-6
NWARM = 8           # junk matmuls to ramp the PE clock during DMA wait

# context-chunk pieces for DMA (chunk ranges, sized so early data lands fast)
YPIECES = [(0, 4), (4, 8), (8, 16), (16, 24), (24, 32)]


def _build_nc(zero_bias):
    nc = bacc.Bacc("TRN2", target_bir_lowering=False)

    # --- DRAM I/O (packed on host; see prep_in_maps) ---
    d_x = nc.dram_tensor("x_mm", (128, 2 * QS), MMDT, kind="ExternalInput")
    d_g = nc.dram_tensor("g_mm", (128, 2 * C), MMDT, kind="ExternalInput")
    # y channel-major, piece p holds chunks [a,b): cols cc*(b-a)*128 + local
    d_y = [nc.dram_tensor(f"y{p}", (128, 2 * (b - a) * 128), MMDT,
                          kind="ExternalInput")
           for p, (a, b) in enumerate(YPIECES)]
    # y token-major, piece p: [128, (b-a)*C]
    d_yt = [nc.dram_tensor(f"yt{p}", (128, (b - a) * C), MMDT,
                           kind="ExternalInput")
            for p, (a, b) in enumerate(YPIECES)]
    # w: [b_t (2*C) | w1_t (2*HID) | w2_t (4*C)]
    d_w = nc.dram_tensor("w", (128, 2 * C + 2 * HID + 4 * C), MMDT,
                         kind="ExternalInput")
    # f32 smalls: [b1p (4) | bvec0 (3) | bvec1 (3)]
    d_f = nc.dram_tensor("fv", (128, 10), F32, kind="ExternalInput")
    d_out = nc.dram_tensor("out_sh", (C, QS), F32, kind="ExternalOutput")

    tabs = list(get_activation_tables(nc.m.arch).keys())
    LNEXP_SET = tabs.index("natural_log_exp_and_others")

    with tile.TileContext(nc) as tc:
        # Pre-load the exp+ln activation table once so the auto-inserted loads
        # don't ping-pong between exp-only and ln-only sets mid-kernel.
        nc.scalar.add_instruction(mybir.InstLoadActFuncSet(
            name=nc.get_next_instruction_name(), ins=[], outs=[],
            act_func_set_id=LNEXP_SET))

        with (
            tc.tile_pool(name="sb", bufs=1) as sb,
            tc.tile_pool(name="pt_pool", bufs=3) as ptp,
            tc.tile_pool(name="scp", bufs=2, space="PSUM") as scp,
            tc.tile_pool(name="ps", bufs=1, space="PSUM") as ps,
        ):
            # -------- input DMAs --------
            # sync and gpsimd are 16-engine striped queues (~150-300 GB/s);
            # the scalar queue is single-engine (~60 GB/s) -> tiny/late only.
            xm = [sb.tile([128, QS], MMDT, name=f"xm{i}") for i in range(2)]
            g_t = sb.tile([128, 2 * C], MMDT)
            yq = [sb.tile([128, 2 * (b - a) * 128], MMDT, name=f"yq{p}")
                  for p, (a, b) in enumerate(YPIECES)]
            ytt = [sb.tile([128, (b - a) * C], MMDT, name=f"ytt{p}")
                   for p, (a, b) in enumerate(YPIECES)]
            fv = sb.tile([128, 10], F32)
            wt = sb.tile([128, 2 * C + 2 * HID + 4 * C], MMDT)

            nc.sync.dma_start(xm[0], d_x[:, 0:QS])
            nc.sync.dma_start(yq[0], d_y[0][:, :])
            nc.sync.dma_start(ytt[0], d_yt[0][:, :])
            nc.sync.dma_start(yq[2], d_y[2][:, :])
            nc.sync.dma_start(ytt[2], d_yt[2][:, :])
            nc.scalar.dma_start(g_t, d_g[:, :])
            nc.scalar.dma_start(fv, d_f[:, :])
            nc.scalar.dma_start(ytt[4], d_yt[4][:, :])
            nc.gpsimd.dma_start(xm[1], d_x[:, QS:2 * QS])
            nc.gpsimd.dma_start(yq[1], d_y[1][:, :])
            nc.gpsimd.dma_start(ytt[1], d_yt[1][:, :])
            nc.gpsimd.dma_start(yq[3], d_y[3][:, :])
            nc.gpsimd.dma_start(yq[4], d_y[4][:, :])
            nc.gpsimd.dma_start(ytt[3], d_yt[3][:, :])
            nc.gpsimd.dma_start(wt, d_w[:, :])

            b_t = wt[:, 0:2 * C]
            w1_t = wt[:, 2 * C:2 * C + 2 * HID]
            w2_t = wt[:, 2 * C + 2 * HID:2 * C + 2 * HID + 4 * C]
            b1p = fv[:, 0:4]
            bvec = [fv[:, 4:7], fv[:, 7:10]]

            ones_c = sb.tile([128, 2], MMDT)
            nc.vector.memset(ones_c, 1.0)
            ones_r = sb.tile([1, 128], MMDT)
            nc.vector.memset(ones_r, 1.0)
            eps2v = sb.tile([1, 1], F32)
            nc.vector.memset(eps2v, float(C) * float(C) * EPS)
            lnCv = sb.tile([1, 1], F32)
            nc.vector.memset(lnCv, math.log(float(C)))
            junk = sb.tile([128, 512], MMDT)
            nc.vector.memset(junk, 0.0)

            # -------- PE clock-gate warmup on junk data during DMA wait ------
            for w in range(NWARM):
                wup = ps.tile([2, 512], F32, tag="st0", name=f"wup{w}")
                nc.tensor.matmul(wup, ones_c, junk, start=True, stop=True)

            def wsl(t, cc, cb, w=128):
                # packed weight tile slice: row-chunk cc, col-chunk cb
                return t[:, cc * (t.shape[1] // 2) + cb * w:
                         cc * (t.shape[1] // 2) + (cb + 1) * w]

            def piece(i):
                for p, (a, b) in enumerate(YPIECES):
                    if a <= i < b:
                        return p, a, b
                raise ValueError(i)

            def ysl(i, cc):
                # y channel-major stationary slice for context chunk i
                p, a, b = piece(i)
                col = cc * (b - a) * 128 + (i - a) * 128
                return yq[p][:, col:col + 128]

            def ytsl(i, cb):
                # y token-major stationary slice for context chunk i
                p, a, b = piece(i)
                col = (i - a) * C + cb * 128
                return ytt[p][:, col:col + 128]

            # ---------------- Qt = (G x + Wk^T bq)/16, channel-major ---------
            # (the 1/16 softmax scale is folded into G and qb on host)
            qp = [sb.tile([128, QS], MMDT, name=f"qp{i}") for i in range(2)]
            qps = scp.tile([128, 1024], F32, tag="spair", name="qps")
            for cb in range(2):
                nc.tensor.matmul(qps[:, cb * 512:(cb + 1) * 512],
                                 wsl(g_t, 0, cb), xm[0],
                                 start=True, stop=False)
                nc.tensor.matmul(qps[:, cb * 512:(cb + 1) * 512],
                                 wsl(g_t, 1, cb), xm[1],
                                 start=False, stop=True)
                if cb == 0:
                    nc.scalar.activation(qp[cb], qps[:, 0:512], AF.Identity,
                                         bias=bvec[cb][:, 0:1])
                else:
                    # DVE path so both qp copies overlap (junk is zeros)
                    nc.vector.scalar_tensor_tensor(qp[cb], qps[:, 512:1024],
                                                   bvec[cb][:, 0:1], junk,
                                                   op0=ALU.add, op1=ALU.add)

            # ---------------- attention state ----------------
            attps = [ps.tile([128, QS], F32, tag=f"attps{j}", bufs=1,
                             name=f"attps{j}") for j in range(2)]
            csum = None
            if not zero_bias:
                csum = ps.tile([2, QS], F32, tag="st1", bufs=1)

            att_s = [sb.tile([128, QS], MMDT, name=f"att_s{i}") for i in range(2)]
            zs = [sb.tile([128, QS], MMDT, name=f"zs{i}") for i in range(2)]
            zsq = [sb.tile([128, QS], MMDT, name=f"zsq{i}") for i in range(2)]
            zln = [sb.tile([128, QS], MMDT, name=f"zln{i}") for i in range(2)]
            hs = [sb.tile([128, QS], MMDT, name=f"hs{i}") for i in range(4)]
            ot = [sb.tile([128, QS], F32, name=f"ot{i}") for i in range(2)]
            rstd = sb.tile([1, QS], MMDT)
            nmrs = sb.tile([1, QS], MMDT)
            nmC = sb.tile([1, QS], F32)
            s2c = sb.tile([1, QS], F32)
            var = sb.tile([1, QS], F32)
            lnv = sb.tile([1, QS], F32)

            def attn_score(p):
                """S^T and exp for context chunk pair (2p, 2p+1)"""
                sps = scp.tile([128, 1024], F32, tag="spair", name=f"sps{p}")
                for k in range(2):
                    i = 2 * p + k
                    nc.tensor.matmul(sps[:, k * 512:(k + 1) * 512],
                                     ysl(i, 0), qp[0], start=True, stop=False)
                    nc.tensor.matmul(sps[:, k * 512:(k + 1) * 512],
                                     ysl(i, 1), qp[1], start=False, stop=True)
                pt = ptp.tile([128, 1024], MMDT, tag="pt", name=f"pt{p}")
                nc.scalar.activation(pt, sps, AF.Exp)
                return pt

            def attn_accum(p, pt):
                """(y P) [and colsum] accumulation for chunk pair (2p, 2p+1)"""
                for k in range(2):
                    i = 2 * p + k
                    first, last = (i == 0), (i == NBLK - 1)
                    pts = pt[:, k * 512:(k + 1) * 512]
                    for cb in range(2):
                        nc.tensor.matmul(attps[cb], ytsl(i, cb), pts,
                                         start=first, stop=last)
                    if not zero_bias:
                        nc.tensor.matmul(csum, ones_c, pts, start=first, stop=last)

            # ---- attention, software-pipelined by one chunk pair so the exp
            # ---- latency sits off the PE's static instruction order ----
            prev = attn_score(0)
            for p in range(1, NPAIR):
                cur = attn_score(p)
                attn_accum(p - 1, prev)
                prev = cur
            attn_accum(NPAIR - 1, prev)

            if zero_bias:
                # LN absorbs the softmax denominator: att_s = yP as-is.
                # Split the two copies across ACT and DVE so they overlap.
                nc.scalar.copy(att_s[0], attps[0])
                nc.vector.tensor_copy(att_s[1], attps[1])
            else:
                # softmax normalize: 1/colsum via exp(-ln(x)) on ACT
                lncs = sb.tile([1, QS], F32)
                rr = sb.tile([1, QS], MMDT)
                rb_s = sb.tile([128, QS], F32)
                nc.scalar.activation(lncs, csum[0:1, :], AF.Ln)
                nc.scalar.activation(rr, lncs, AF.Exp, scale=-1.0)
                rb = ps.tile([128, QS], F32, tag="st0", name="rb")
                nc.tensor.matmul(rb, ones_r, rr, start=True, stop=True)
                nc.vector.tensor_copy(rb_s, rb)
                for cb in range(2):
                    nc.vector.scalar_tensor_tensor(att_s[cb], attps[cb], 1.0,
                                                   rb_s, op0=ALU.mult,
                                                   op1=ALU.mult)

            # z = B @ att_s + bo'
            zps = scp.tile([128, 1024], F32, tag="spair", name="zps")
            for cb in range(2):
                nc.tensor.matmul(zps[:, cb * 512:(cb + 1) * 512],
                                 wsl(b_t, 0, cb), att_s[0], start=True, stop=False)
                nc.tensor.matmul(zps[:, cb * 512:(cb + 1) * 512],
                                 wsl(b_t, 1, cb), att_s[1], start=False, stop=True)

            HQ = QS // 2
            sls = [slice(0, HQ), slice(HQ, QS)]

            if zero_bias:
                # zs (DVE copy) and zsq (ACT square) per column half, so the
                # LN stat chain pipelines across engines
                for h in range(2):
                    for cb in range(2):
                        zph = zps[:, cb * 512 + h * HQ: cb * 512 + (h + 1) * HQ]
                        nc.vector.tensor_copy(zs[cb][:, sls[h]], zph)
                        nc.vector.tensor_mul(zsq[cb][:, sls[h]],
                                             zs[cb][:, sls[h]],
                                             zs[cb][:, sls[h]])
            else:
                for cb in range(2):
                    zph = zps[:, cb * 512:(cb + 1) * 512]
                    nc.scalar.activation(zs[cb], zph, AF.Identity,
                                         bias=bvec[cb][:, 1:2])
                    nc.vector.tensor_mul(zsq[cb], zs[cb], zs[cb])

            # LN stats per column half in the freed attps banks
            szp, sqp = {}, {}
            for h in range(2):
                szp[h] = ps.tile([2, HQ], F32, tag="attps0", name=f"szp{h}")
                nc.tensor.matmul(szp[h], ones_c, zs[0][:, sls[h]],
                                 start=True, stop=False)
                nc.tensor.matmul(szp[h], ones_c, zs[1][:, sls[h]],
                                 start=False, stop=True)
                sqp[h] = ps.tile([2, HQ], F32, tag="attps1", name=f"sqp{h}")
                nc.tensor.matmul(sqp[h], ones_c, zsq[0][:, sls[h]],
                                 start=True, stop=False)
                nc.tensor.matmul(sqp[h], ones_c, zsq[1][:, sls[h]],
                                 start=False, stop=True)

            # var*C^2 = C*sum(z^2) - (sum z)^2; rstd = C/sqrt(var*C^2 + C^2 eps)
            for h in range(2):
                sl = sls[h]
                nc.vector.tensor_scalar_mul(nmC[:, sl], szp[h][0:1, :], -1.0)
                nc.vector.tensor_mul(s2c[:, sl], nmC[:, sl], nmC[:, sl])
                nc.vector.scalar_tensor_tensor(var[:, sl], sqp[h][0:1, :],
                                               float(C), s2c[:, sl],
                                               op0=ALU.mult, op1=ALU.subtract)
                nc.scalar.activation(lnv[:, sl], var[:, sl], AF.Ln, bias=eps2v)
                nc.scalar.activation(rstd[:, sl], lnv[:, sl], AF.Exp,
                                     scale=-0.5, bias=lnCv)
                nc.vector.scalar_tensor_tensor(nmrs[:, sl], nmC[:, sl], 1.0 / C,
                                               rstd[:, sl], op0=ALU.mult,
                                               op1=ALU.mult)

            zt = [sb.tile([128, QS], MMDT, name=f"zt{cb}") for cb in range(2)]
            for h in range(2):
                sl = sls[h]
                rstd_b = ps.tile([128, HQ], F32, tag="attps0", name=f"rstdb{h}")
                nc.tensor.matmul(rstd_b, ones_r, rstd[:, sl], start=True, stop=True)
                nmrs_b = ps.tile([128, HQ], F32, tag="attps1", name=f"nmrsb{h}")
                nc.tensor.matmul(nmrs_b, ones_r, nmrs[:, sl], start=True, stop=True)
                for cb in range(2):
                    nc.vector.tensor_mul(zt[cb][:, sl], zs[cb][:, sl], rstd_b)
                    nc.vector.tensor_add(zln[cb][:, sl], zt[cb][:, sl], nmrs_b)

            # MLP + residual
            for hp in range(2):
                hps = scp.tile([128, 1024], F32, tag="spair", name=f"hps{hp}")
                for k in range(2):
                    hb = 2 * hp + k
                    nc.tensor.matmul(hps[:, k * 512:(k + 1) * 512],
                                     wsl(w1_t, 0, hb), zln[0], start=True, stop=False)
                    nc.tensor.matmul(hps[:, k * 512:(k + 1) * 512],
                                     wsl(w1_t, 1, hb), zln[1], start=False, stop=True)
                    nc.scalar.activation(hs[hb], hps[:, k * 512:(k + 1) * 512],
                                         AF.Gelu, bias=b1p[:, hb:hb + 1])

            tps = scp.tile([128, 1024], F32, tag="spair", name="tps")
            for cb in range(2):
                for hb in range(4):
                    nc.tensor.matmul(
                        tps[:, cb * 512:(cb + 1) * 512],
                        w2_t[:, hb * 256 + cb * 128:hb * 256 + (cb + 1) * 128],
                        hs[hb], start=(hb == 0), stop=(hb == 3))
            # out = mlp + b2 + residual, quartered so DMA-out starts early
            for h in range(2):
                for cb in range(2):
                    nc.vector.scalar_tensor_tensor(
                        ot[cb][:, sls[h]],
                        tps[:, cb * 512 + h * HQ: cb * 512 + (h + 1) * HQ],
                        bvec[cb][:, 2:3], xm[cb][:, h * HQ:
                                              (h + 1) * HQ],
                        op0=ALU.add, op1=ALU.add)
                    q = nc.sync if cb == 0 else nc.gpsimd
                    q.dma_start(d_out[cb * 128:(cb + 1) * 128, h * HQ:(h + 1) * HQ],
                                ot[cb][:, sls[h]])

    nc.compile()
    return nc


_NCS = {}


def _get_nc(zero_bias=True):
    if zero_bias not in _NCS:
        _NCS[zero_bias] = _build_nc(zero_bias)
    return _NCS[zero_bias]


def _pack_rows(a, nchunk):
    """(nchunk*128, W) -> (128, nchunk*W) with row-chunks side by side."""
    w = a.shape[1]
    out = np.empty((128, nchunk * w), a.dtype)
    for i in range(nchunk):
        out[:, i * w:(i + 1) * w] = a[i * 128:(i + 1) * 128, :]
    return out


def prep_in_maps(x, y, Wq, bq, Wk, bk, Wv, bv, Wo, bo, ln_w, ln_b, W1, b1, W2, b2):
    f = lambda a: np.asarray(a, dtype=np.float32)
    x, y = f(x), f(y)
    Wq, bq, Wk, Wv, bv, Wo, bo = f(Wq), f(bq), f(Wk), f(Wv), f(bv), f(Wo), f(bo)
    ln_w, ln_b, W1, b1, W2, b2 = f(ln_w), f(ln_b), f(W1), f(b1), f(W2), f(b2)

    mmnp = mybir.dt.np(MMDT)
    g = lambda a: np.ascontiguousarray(a).astype(mmnp)

    x_cm = np.ascontiguousarray(x.reshape(C, HW))
    y_cm = np.ascontiguousarray(y.reshape(C, NCTX))

    # host-side algebraic folds
    G = (Wk.astype(np.float64).T @ Wq.astype(np.float64) / 16.0).astype(np.float32)
    B = (Wo.astype(np.float64) @ Wv.astype(np.float64)).astype(np.float32)
    qb = (Wk.astype(np.float64).T @ bq.astype(np.float64)).astype(np.float32)
    bo_p = (Wo.astype(np.float64) @ bv.astype(np.float64) + bo).astype(np.float32)
    b1_p = (W1.astype(np.float64) @ ln_b.astype(np.float64) + b1).astype(np.float32)
    W1p = (W1 * ln_w[None, :]).astype(np.float32)

    # f32 smalls: [b1p (4) | bvec0 (3) | bvec1 (3)]
    bvec = np.stack([qb / 16.0, bo_p, b2], axis=1).astype(np.float32)  # (256,3)
    fv = np.concatenate([np.ascontiguousarray(b1_p.reshape(4, 128).T),
                         bvec[0:128, :], bvec[128:256, :]], axis=1)

    # y pieces, channel-major: piece p = chunks [a,b): [cc | chunk-local]
    y2 = _pack_rows(y_cm, 2)  # (128, 2*NCTX), cc side by side
    y_pieces = []
    for a, b in YPIECES:
        y_pieces.append(np.concatenate(
            [y2[:, cc * NCTX + a * 128: cc * NCTX + b * 128] for cc in range(2)],
            axis=1))
    # y pieces, token-major: chunk ci = y^T rows [ci*128,(ci+1)*128) = [128, C]
    y_tm = _pack_rows(np.ascontiguousarray(y_cm.T), NBLK)  # (128, NBLK*C)
    yt_pieces = [y_tm[:, a * C: b * C] for a, b in YPIECES]

    wpk = np.concatenate([_pack_rows(B.T, 2), _pack_rows(W1p.T, 2),
                          _pack_rows(W2.T, 4)], axis=1)

    common = {"w": g(wpk), "fv": fv.astype(np.float32),
              "g_mm": g(_pack_rows(G.T, 2))}
    for p in range(len(YPIECES)):
        common[f"y{p}"] = g(y_pieces[p])
        common[f"yt{p}"] = g(yt_pieces[p])

    in_maps = []
    for i in range(NCORES):
        m = dict(common)
        xs = np.ascontiguousarray(x_cm[:, i * QS:(i + 1) * QS])
        m["x_mm"] = g(_pack_rows(xs, 2))
        in_maps.append(m)
    return in_maps


def kernel(**inputs):
    in_maps = prep_in_maps(**inputs)
    f64 = lambda a: np.asarray(a, dtype=np.float64)
    bo_p = f64(inputs["Wo"]) @ f64(inputs["bv"]) + f64(inputs["bo"])
    nc = _get_nc(zero_bias=bool(np.abs(bo_p).max() == 0.0))
    res = bass_utils.run_bass_kernel_spmd(nc, in_maps, core_ids=list(range(NCORES)))
    t = np.concatenate([res.results[i]["out_sh"] for i in range(NCORES)], axis=1)
    return t.reshape(1, C, 64, 64)
